# revision 15
# baseline (speedup 1.0000x reference)
"""Trainium2 Bass kernel for nn_DetectLayer (YOLO-style detect layer).

Contract: kernel(**inputs) takes FULL unsharded inputs (nB=8 images) and
returns (p_bbox, cls_idx, score, loss) matching reference.reference().

Sharding: pure data parallel — one image per NeuronCore (8 cores).
Each core:
  - decodes p_bbox from t_bbox (sigmoid on ACT, affine on DVE)
  - max/argmax over 80 classes (DVE tensor_reduce + max_index)
  - score = sigmoid(conf) * sigmoid(max_cls_logit)
  - tconf = max IoU of each decoded box vs the 32 GT boxes, computed in
    log-domain (ln/exp on ACT) to avoid per-GT division
  - loss partial = sum softplus(conf) - sum conf*tconf + gathered small
    losses (xy/wh/cls BCE at matched GT cells, via indirect DMA gather)
Host: slices inputs per image, precomputes tiny per-GT tables (anchor
matching, targets, corner boxes), gathers outputs and sums the 8 loss
partials / nB.

Positions are laid out "p-major": flat position n = p*600 + f, where
p in [0,128) is the SBUF partition and f in [0,600) the free index.
This makes every big DMA per-partition contiguous.
"""

import os
import sys

for _p in ("/opt/trn_rl_repo", "/root/.axon_site/_ro/trn_rl_repo"):
    if os.path.isdir(_p) and _p not in sys.path:
        sys.path.append(_p)

import numpy as np

import concourse.bacc as bacc
import concourse.bass as bass
import concourse.mybir as mybir
import concourse.tile as tile
from concourse.bass_utils import run_bass_kernel_spmd

F32 = mybir.dt.float32
BF16 = mybir.dt.bfloat16
U32 = mybir.dt.uint32
I32 = mybir.dt.int32

P = 128          # SBUF partitions
NA = 3
NH = NW = 160
N = NA * NH * NW  # 76800 positions per image
F = N // P        # 600 free elements per partition
NCLS = 80
G = 32            # padded GT count
NCORES = 8
KC = 50           # cls positions-per-partition per chunk
NCHUNK = F // KC  # 12
STRIDE = 8.0

ANCHORS_ALL = np.array(
    [[10., 13.], [16., 30.], [33., 23.], [30., 61.], [62., 45.],
     [59., 119.], [116., 90.], [156., 198.], [373., 326.]], dtype=np.float32)
ANCH_LVL = ANCHORS_ALL[0:3]

AF = mybir.ActivationFunctionType
OP = mybir.AluOpType
AX = mybir.AxisListType


def _position_planes():
    """Image-independent per-position planes in p-major flat order.

    n = a*25600 + j*160 + i ; returns [4, N] f32: 8*i-4, 8*j-4, anch_w, anch_h
    (the -4 folds the reference's (2s-0.5+grid)*8 = 16s + (8*grid - 4)).
    """
    n = np.arange(N, dtype=np.int64)
    i = (n % NW).astype(np.float32)
    j = ((n // NW) % NH).astype(np.float32)
    a = (n // (NH * NW)).astype(np.int64)
    g8x = 8.0 * i - 4.0
    g8y = 8.0 * j - 4.0
    aw = ANCH_LVL[a, 0]
    ah = ANCH_LVL[a, 1]
    return np.stack([g8x, g8y, aw, ah]).astype(np.float32)


def _emit(nc, tc, d):
    """Emit the per-core Tile program. d: dict of DRAM APs."""
    v = nc.vector
    s = nc.scalar
    g = nc.gpsimd
    sy = nc.sync

    from contextlib import ExitStack
    ctx = ExitStack()
    cp = ctx.enter_context(tc.tile_pool(name="const", bufs=1))
    clsp = ctx.enter_context(tc.tile_pool(name="cls", bufs=3))
    pp = ctx.enter_context(tc.tile_pool(name="psum", bufs=1, space="PSUM"))

    # ---------------- load small inputs ----------------
    tb = cp.tile([P, F, 4], F32, tag="tb")
    sy.dma_start(out=tb[:], in_=d["tbox"].rearrange("(p f) c -> p f c", p=P))
    conf = cp.tile([P, F], F32, tag="conf")
    sy.dma_start(out=conf[:], in_=d["conf"].rearrange("(p f) -> p f", p=P))
    planes = cp.tile([P, 4, F], F32, tag="planes")
    for k in range(4):
        sy.dma_start(out=planes[:, k, :],
                     in_=d["planes"][k].rearrange("(p f) -> p f", p=P))
    g8x, g8y, awp, ahp = (planes[:, k, :] for k in range(4))
    gt = cp.tile([P, 5, G], F32, tag="gt")
    sy.dma_start(out=gt[:], in_=d["gt"][:])
    sl_idx = cp.tile([P, 1], I32, tag="sl_idx")
    sy.dma_start(out=sl_idx[:], in_=d["sl_idx"][:])
    sl_tgt = cp.tile([P, 84], F32, tag="sl_tgt")
    sy.dma_start(out=sl_tgt[:], in_=d["sl_tgt"][:])
    sl_w = cp.tile([P, 2], F32, tag="sl_w")
    sy.dma_start(out=sl_w[:], in_=d["sl_w"][:])

    # ---------------- bbox decode ----------------
    sig = cp.tile([P, F, 4], F32, tag="sig")
    s.activation(out=sig[:], in_=tb[:], func=AF.Sigmoid)
    pbb = cp.tile([P, F, 4], F32, tag="pbb")
    t1 = cp.tile([P, F], F32, tag="t1")
    t2 = cp.tile([P, F], F32, tag="t2")
    # p_xy = 16*sig + (8*grid - 4)
    v.tensor_scalar_mul(pbb[:, :, 0], sig[:, :, 0], 16.0)
    v.tensor_add(pbb[:, :, 0], pbb[:, :, 0], g8x)
    v.tensor_scalar_mul(pbb[:, :, 1], sig[:, :, 1], 16.0)
    v.tensor_add(pbb[:, :, 1], pbb[:, :, 1], g8y)
    # p_wh = (2*sig)^2 * anchor
    s.activation(out=t1[:], in_=sig[:, :, 2], func=AF.Square, scale=2.0)
    v.tensor_mul(pbb[:, :, 2], t1[:], awp)
    s.activation(out=t2[:], in_=sig[:, :, 3], func=AF.Square, scale=2.0)
    v.tensor_mul(pbb[:, :, 3], t2[:], ahp)
    sy.dma_start(out=d["pbb"].rearrange("(p f) c -> p f c", p=P), in_=pbb[:])

    # corners + area (fp32; the iw/ih cancellation needs fp32)
    hw = cp.tile([P, F], F32, tag="hw")
    hh = cp.tile([P, F], F32, tag="hh")
    v.tensor_scalar_mul(hw[:], pbb[:, :, 2], 0.5)
    v.tensor_scalar_mul(hh[:], pbb[:, :, 3], 0.5)
    ax1 = cp.tile([P, F], F32, tag="ax1")
    ax2 = cp.tile([P, F], F32, tag="ax2")
    ay1 = cp.tile([P, F], F32, tag="ay1")
    ay2 = cp.tile([P, F], F32, tag="ay2")
    v.tensor_sub(ax1[:], pbb[:, :, 0], hw[:])
    v.tensor_add(ax2[:], pbb[:, :, 0], hw[:])
    v.tensor_sub(ay1[:], pbb[:, :, 1], hh[:])
    v.tensor_add(ay2[:], pbb[:, :, 1], hh[:])
    area_b = cp.tile([P, F], BF16, tag="area_b")
    v.tensor_mul(t1[:], pbb[:, :, 2], pbb[:, :, 3])
    v.tensor_copy(out=area_b[:], in_=t1[:])

    # ---------------- tconf: max IoU vs 32 GTs (log domain) ----------------
    best = cp.tile([P, F], BF16, tag="best")
    v.memset(best[:], -1e30)
    epsb = cp.tile([P, 1], F32, tag="epsb")
    v.memset(epsb[:], 1e-30)
    xa = cp.tile([P, F], F32, tag="xa")
    xb = cp.tile([P, F], F32, tag="xb")
    iwb = cp.tile([P, F], BF16, tag="iwb")
    ihb = cp.tile([P, F], BF16, tag="ihb")
    inter = cp.tile([P, F], BF16, tag="inter")
    unio = cp.tile([P, F], BF16, tag="unio")
    li = cp.tile([P, F], BF16, tag="li")
    lu = cp.tile([P, F], BF16, tag="lu")
    for gi in range(G):
        # iw = relu(min(ax2,bx2) - max(ax1,bx1))  [fp32 -> bf16 via ACT relu]
        v.tensor_scalar_min(xa[:], ax2[:], gt[:, 1, gi:gi + 1])
        v.tensor_scalar_max(xb[:], ax1[:], gt[:, 0, gi:gi + 1])
        v.tensor_sub(xa[:], xa[:], xb[:])
        s.activation(out=iwb[:], in_=xa[:], func=AF.Relu)
        v.tensor_scalar_min(xa[:], ay2[:], gt[:, 3, gi:gi + 1])
        v.tensor_scalar_max(xb[:], ay1[:], gt[:, 2, gi:gi + 1])
        v.tensor_sub(xa[:], xa[:], xb[:])
        s.activation(out=ihb[:], in_=xa[:], func=AF.Relu)
        v.tensor_mul(inter[:], iwb[:], ihb[:])
        # union = area + areaG - inter
        v.tensor_scalar_add(unio[:], area_b[:], gt[:, 4, gi:gi + 1])
        v.tensor_sub(unio[:], unio[:], inter[:])
        # d = ln(inter + eps) - ln(union); best = max(best, d)
        s.activation(out=li[:], in_=inter[:], func=AF.Ln, bias=epsb[:])
        s.activation(out=lu[:], in_=unio[:], func=AF.Ln)
        v.tensor_sub(li[:], li[:], lu[:])
        v.tensor_max(best[:], best[:], li[:])
    tconf = cp.tile([P, F], F32, tag="tconf")
    s.activation(out=tconf[:], in_=best[:], func=AF.Exp)

    # ---------------- cls max / argmax + score ----------------
    maxv = cp.tile([P, F], F32, tag="maxv")
    idx8 = cp.tile([P, F, 8], U32, tag="idx8")
    cls_r = d["cls"].rearrange("(p f) c -> p f c", p=P)
    for c in range(NCHUNK):
        ct = clsp.tile([P, KC, NCLS], F32, tag="clsbuf")
        sy.dma_start(out=ct[:], in_=cls_r[:, c * KC:(c + 1) * KC, :])
        v.reduce_max(out=maxv[:, c * KC:(c + 1) * KC], in_=ct[:], axis=AX.X)
        for k in range(KC):
            j = c * KC + k
            v.max_index(out=idx8[:, j, :],
                        in_max=maxv[:, j:j + 1].to_broadcast([P, 8]),
                        in_values=ct[:, k, :])
    idxp = cp.tile([P, F], U32, tag="idxp")
    v.tensor_copy(out=idxp[:], in_=idx8[:, :, 0])
    sy.dma_start(out=d["idx"].rearrange("(p f) -> p f", p=P), in_=idxp[:])

    smx = cp.tile([P, F], F32, tag="smx")
    s.activation(out=smx[:], in_=maxv[:], func=AF.Sigmoid)
    scf = cp.tile([P, F], F32, tag="scf")
    s.activation(out=scf[:], in_=conf[:], func=AF.Sigmoid)
    sco = cp.tile([P, F], F32, tag="sco")
    v.tensor_mul(sco[:], smx[:], scf[:])
    sy.dma_start(out=d["sco"].rearrange("(p f) -> p f", p=P), in_=sco[:])

    # ---------------- loss ----------------
    # dense part: sum softplus(conf) - sum conf*tconf
    # softplus(x) = ln(1 + e^x); safe since |conf logits| <~ 6.
    sp_acc = cp.tile([P, 1], F32, tag="sp_acc")
    spt = cp.tile([P, F], F32, tag="spt")
    s.activation(out=spt[:], in_=conf[:], func=AF.Exp)
    v.tensor_scalar_add(spt[:], spt[:], 1.0)
    s.activation(out=spt[:], in_=spt[:], func=AF.Ln, accum_out=sp_acc[:])
    v.tensor_mul(spt[:], conf[:], tconf[:])
    dot = cp.tile([P, 1], F32, tag="dot")
    v.reduce_sum(out=dot[:], in_=spt[:], axis=AX.X)

    # small losses at the 32 matched cells (gather rows via indirect DMA)
    gb4 = cp.tile([P, 4], F32, tag="gb4")
    g.indirect_dma_start(
        out=gb4[:], out_offset=None, in_=d["tbox"][:],
        in_offset=bass.IndirectOffsetOnAxis(ap=sl_idx[:, :1], axis=0))
    gb80 = cp.tile([P, NCLS], F32, tag="gb80")
    g.indirect_dma_start(
        out=gb80[:], out_offset=None, in_=d["cls"][:],
        in_offset=bass.IndirectOffsetOnAxis(ap=sl_idx[:, :1], axis=0))
    bce = cp.tile([P, 84], F32, tag="bce")
    xt = cp.tile([P, 84], F32, tag="xt")
    s.activation(out=bce[:, 0:4], in_=gb4[:], func=AF.Exp)
    s.activation(out=bce[:, 4:84], in_=gb80[:], func=AF.Exp)
    v.tensor_scalar_add(bce[:], bce[:], 1.0)
    s.activation(out=bce[:], in_=bce[:], func=AF.Ln)
    v.tensor_mul(xt[:, 0:4], gb4[:], sl_tgt[:, 0:4])
    v.tensor_mul(xt[:, 4:84], gb80[:], sl_tgt[:, 4:84])
    v.tensor_sub(bce[:], bce[:], xt[:])
    r1 = cp.tile([P, 1], F32, tag="r1")
    r2 = cp.tile([P, 1], F32, tag="r2")
    v.reduce_sum(out=r1[:], in_=bce[:, 0:4], axis=AX.X)
    v.reduce_sum(out=r2[:], in_=bce[:, 4:84], axis=AX.X)
    v.tensor_scalar_mul(r1[:], r1[:], sl_w[:, 0:1])
    v.tensor_scalar_mul(r2[:], r2[:], sl_w[:, 1:2])

    tot = cp.tile([P, 1], F32, tag="tot")
    v.tensor_sub(tot[:], sp_acc[:], dot[:])
    v.tensor_add(tot[:], tot[:], r1[:])
    v.tensor_add(tot[:], tot[:], r2[:])
    ones = cp.tile([P, 1], F32, tag="ones")
    v.memset(ones[:], 1.0)
    lps = pp.tile([1, 1], F32, tag="lps")
    nc.tensor.matmul(out=lps[:], lhsT=tot[:], rhs=ones[:], start=True, stop=True)
    lsb = cp.tile([1, 1], F32, tag="lsb")
    v.tensor_copy(out=lsb[:], in_=lps[:])
    sy.dma_start(out=d["los"][:], in_=lsb[:])
    ctx.close()


_NC = None


def build_nc():
    global _NC
    if _NC is not None:
        return _NC
    nc = bacc.Bacc("TRN2", target_bir_lowering=False)
    d = {}
    d["cls"] = nc.dram_tensor("cls", [N, NCLS], F32, kind="ExternalInput").ap()
    d["tbox"] = nc.dram_tensor("tbox", [N, 4], F32, kind="ExternalInput").ap()
    d["conf"] = nc.dram_tensor("conf", [N], F32, kind="ExternalInput").ap()
    d["planes"] = nc.dram_tensor("planes", [4, N], F32, kind="ExternalInput").ap()
    d["gt"] = nc.dram_tensor("gt", [P, 5, G], F32, kind="ExternalInput").ap()
    d["sl_idx"] = nc.dram_tensor("sl_idx", [P, 1], I32, kind="ExternalInput").ap()
    d["sl_tgt"] = nc.dram_tensor("sl_tgt", [P, 84], F32, kind="ExternalInput").ap()
    d["sl_w"] = nc.dram_tensor("sl_w", [P, 2], F32, kind="ExternalInput").ap()
    d["pbb"] = nc.dram_tensor("pbb", [N, 4], F32, kind="ExternalOutput").ap()
    d["idx"] = nc.dram_tensor("idx", [N], U32, kind="ExternalOutput").ap()
    d["sco"] = nc.dram_tensor("sco", [N], F32, kind="ExternalOutput").ap()
    d["los"] = nc.dram_tensor("los", [1, 1], F32, kind="ExternalOutput").ap()
    with tile.TileContext(nc) as tc:
        _emit(nc, tc, d)
    nc.compile()
    _NC = nc
    return nc


def make_in_maps(t_bbox, conf_logits, cls_logits, gt_bboxes, gt_cls, gt_mask):
    planes = _position_planes()
    t_bbox = np.asarray(t_bbox, dtype=np.float32)
    conf_logits = np.asarray(conf_logits, dtype=np.float32)
    cls_logits = np.asarray(cls_logits, dtype=np.float32)
    gt_bboxes = np.asarray(gt_bboxes, dtype=np.float32)
    gt_cls = np.asarray(gt_cls)
    gt_mask = np.asarray(gt_mask)

    # ---- host-side per-GT matching (tiny, mirrors reference exactly) ----
    gw, gh = gt_bboxes[..., 2], gt_bboxes[..., 3]          # [nB,G]
    aw, ah = ANCHORS_ALL[:, 0], ANCHORS_ALL[:, 1]          # [9]
    inter = np.minimum(gw[..., None], aw) * np.minimum(gh[..., None], ah)
    union = (gw * gh)[..., None] + aw * ah - inter
    anch_idx = np.argmax(inter / (union + 1e-16), axis=-1)  # [nB,G]
    matched = anch_idx < NA
    valid = gt_mask & matched
    ta = (anch_idx % NA).astype(np.int64)
    ti = np.clip((gt_bboxes[..., 0] / STRIDE).astype(np.int32), 0, NW - 1)
    tj = np.clip((gt_bboxes[..., 1] / STRIDE).astype(np.int32), 0, NH - 1)
    flat = (ta * (NH * NW) + tj.astype(np.int64) * NW + ti).astype(np.int32)
    tgt_xy = ((gt_bboxes[..., 0:2] / STRIDE) % 1.0 + 0.5) / 2.0
    tgt_wh = np.sqrt(gt_bboxes[..., 2:4] / ANCH_LVL[ta]) / 2.0
    onehot = (gt_cls[..., None] == np.arange(NCLS)).astype(np.float32)
    vm = valid.astype(np.float32)

    # GT corner boxes for tconf IoU; invalid GTs -> far-away unit boxes so
    # their IoU is exactly 0 (matches reference's mask->-1 + has_gt logic).
    cx, cy = gt_bboxes[..., 0], gt_bboxes[..., 1]
    bx1 = cx - gw * 0.5
    bx2 = cx + gw * 0.5
    by1 = cy - gh * 0.5
    by2 = cy + gh * 0.5
    ag = gw * gh
    m = gt_mask.astype(bool)
    FARV = np.float32(4e6)
    bx1 = np.where(m, bx1, FARV)
    bx2 = np.where(m, bx2, FARV + 1)
    by1 = np.where(m, by1, FARV)
    by2 = np.where(m, by2, FARV + 1)
    ag = np.where(m, ag, np.float32(1.0))

    in_maps = []
    for b in range(NCORES):
        gt5 = np.stack([bx1[b], bx2[b], by1[b], by2[b], ag[b]]).astype(np.float32)
        gt_t = np.broadcast_to(gt5[None], (P, 5, G)).copy()
        sl_idx = np.zeros((P, 1), np.int32)
        sl_idx[:G, 0] = np.where(valid[b], flat[b], 0)
        sl_tgt = np.zeros((P, 84), np.float32)
        sl_tgt[:G, 0:2] = tgt_xy[b]
        sl_tgt[:G, 2:4] = tgt_wh[b]
        sl_tgt[:G, 4:84] = onehot[b]
        sl_w = np.zeros((P, 2), np.float32)
        sl_w[:G, 0] = vm[b]
        sl_w[:G, 1] = vm[b] / np.float32(NCLS)
        in_maps.append(dict(
            cls=np.ascontiguousarray(cls_logits[b].reshape(N, NCLS)),
            tbox=np.ascontiguousarray(t_bbox[b].reshape(N, 4)),
            conf=np.ascontiguousarray(conf_logits[b].reshape(N)),
            planes=planes,
            gt=gt_t,
            sl_idx=sl_idx,
            sl_tgt=sl_tgt,
            sl_w=sl_w,
        ))
    return in_maps


def kernel(t_bbox, conf_logits, cls_logits, gt_bboxes, gt_cls, gt_mask):
    nc = build_nc()
    in_maps = make_in_maps(t_bbox, conf_logits, cls_logits,
                           gt_bboxes, gt_cls, gt_mask)
    res = run_bass_kernel_spmd(nc, in_maps, list(range(NCORES)))
    rs = res.results
    p_bbox = np.stack([rs[b]["pbb"] for b in range(NCORES)])
    cls_idx = np.stack([rs[b]["idx"].view(np.int32) for b in range(NCORES)])
    score = np.stack([rs[b]["sco"] for b in range(NCORES)])
    loss = np.float32(sum(float(rs[b]["los"][0, 0]) for b in range(NCORES)) / NCORES)
    return p_bbox, cls_idx, score, loss


# revision 31
# speedup vs baseline: 13.2069x; 13.2069x over previous
"""Trainium2 Bass kernel for nn_DetectLayer (YOLO-style detect layer).

Contract: kernel(**inputs) takes FULL unsharded inputs (nB=8 images) and
returns (p_bbox, cls_idx, score, loss) matching reference.reference().

Sharding: pure data parallel — one image per NeuronCore (8 cores).
Each core:
  - decodes p_bbox from t_bbox (sigmoid on ACT, affine on DVE)
  - max/argmax over 80 classes (DVE tensor_reduce + max_index)
  - score = sigmoid(conf) * sigmoid(max_cls_logit)
  - tconf = max IoU of each decoded box vs the 32 GT boxes, computed in
    log-domain (ln/exp on ACT) to avoid per-GT division
  - loss partial = sum softplus(conf) - sum conf*tconf + gathered small
    losses (xy/wh/cls BCE at matched GT cells, via indirect DMA gather)
Host: slices inputs per image, precomputes tiny per-GT tables (anchor
matching, targets, corner boxes), gathers outputs and sums the 8 loss
partials / nB.

Positions are laid out "p-major": flat position n = p*600 + f, where
p in [0,128) is the SBUF partition and f in [0,600) the free index.
This makes every big DMA per-partition contiguous.
"""

import os
import sys

for _p in ("/opt/trn_rl_repo", "/root/.axon_site/_ro/trn_rl_repo"):
    if os.path.isdir(_p) and _p not in sys.path:
        sys.path.append(_p)

import numpy as np

import concourse.bacc as bacc
import concourse.bass as bass
import concourse.mybir as mybir
import concourse.tile as tile
from concourse.bass_utils import run_bass_kernel_spmd

F32 = mybir.dt.float32
BF16 = mybir.dt.bfloat16
F16 = mybir.dt.float16
U32 = mybir.dt.uint32
I32 = mybir.dt.int32
IOU_DT = F16  # IoU pipeline dtype: fp16 = 2x DVE rate, ~1px coord rounding

P = 128          # SBUF partitions
NA = 3
NH = NW = 160
N = NA * NH * NW  # 76800 positions per image
F = N // P        # 600 free elements per partition
NCLS = 80
G = 32            # padded GT count
NCORES = 8
KC = 50           # cls positions-per-partition per chunk
NCHUNK = F // KC  # 12
STRIDE = 8.0

ANCHORS_ALL = np.array(
    [[10., 13.], [16., 30.], [33., 23.], [30., 61.], [62., 45.],
     [59., 119.], [116., 90.], [156., 198.], [373., 326.]], dtype=np.float32)
ANCH_LVL = ANCHORS_ALL[0:3]

AF = mybir.ActivationFunctionType
OP = mybir.AluOpType
AX = mybir.AxisListType


def _position_planes():
    """Image-independent per-position planes in p-major flat order.

    n = a*25600 + j*160 + i ; returns [4, N] f32: 8*i-4, 8*j-4, anch_w, anch_h
    (the -4 folds the reference's (2s-0.5+grid)*8 = 16s + (8*grid - 4)).
    """
    n = np.arange(N, dtype=np.int64)
    i = (n % NW).astype(np.float32)
    j = ((n // NW) % NH).astype(np.float32)
    a = (n // (NH * NW)).astype(np.int64)
    g8x = 8.0 * i - 4.0
    g8y = 8.0 * j - 4.0
    aw = ANCH_LVL[a, 0]
    ah = ANCH_LVL[a, 1]
    return np.stack([g8x, g8y, aw, ah]).astype(np.float32)


def _emit(nc, tc, d, sfx=""):
    """Emit the per-core Tile program. d: dict of DRAM APs."""
    v = nc.vector
    s = nc.scalar
    g = nc.gpsimd
    sy = nc.sync

    from contextlib import ExitStack
    ctx = ExitStack()
    cp = ctx.enter_context(tc.tile_pool(name="const" + sfx, bufs=1))
    clsp = ctx.enter_context(tc.tile_pool(name="cls" + sfx, bufs=3))
    pp = ctx.enter_context(tc.tile_pool(name="psum" + sfx, bufs=1, space="PSUM"))

    # ---------------- load small inputs ----------------
    tb = cp.tile([P, F, 4], F32, tag="tb")
    sy.dma_start(out=tb[:], in_=d["tbox"].rearrange("(p f) c -> p f c", p=P))
    conf = cp.tile([P, F], F32, tag="conf")
    sy.dma_start(out=conf[:], in_=d["conf"].rearrange("(p f) -> p f", p=P))
    planes = cp.tile([P, 4, F], F32, tag="planes")
    for k in range(4):
        sy.dma_start(out=planes[:, k, :],
                     in_=d["planes"][k].rearrange("(p f) -> p f", p=P))
    g8x, g8y, awp, ahp = (planes[:, k, :] for k in range(4))
    gt = cp.tile([P, 7, G], F32, tag="gt")
    sy.dma_start(out=gt[:], in_=d["gt"][:])
    sl_idx = cp.tile([P, 1], I32, tag="sl_idx")
    sy.dma_start(out=sl_idx[:], in_=d["sl_idx"][:])
    sl_tgt = cp.tile([P, 84], F32, tag="sl_tgt")
    sy.dma_start(out=sl_tgt[:], in_=d["sl_tgt"][:])
    sl_w = cp.tile([P, 2], F32, tag="sl_w")
    sy.dma_start(out=sl_w[:], in_=d["sl_w"][:])

    # ---------------- bbox decode ----------------
    sig = cp.tile([P, F, 4], F32, tag="sig")
    s.activation(out=sig[:], in_=tb[:], func=AF.Sigmoid)
    pbb = cp.tile([P, F, 4], F32, tag="pbb")
    t1 = cp.tile([P, F], F32, tag="t1")
    t2 = cp.tile([P, F], F32, tag="t2")
    # p_xy = 16*sig + (8*grid - 4)
    v.tensor_scalar_mul(pbb[:, :, 0], sig[:, :, 0], 16.0)
    v.tensor_add(pbb[:, :, 0], pbb[:, :, 0], g8x)
    v.tensor_scalar_mul(pbb[:, :, 1], sig[:, :, 1], 16.0)
    v.tensor_add(pbb[:, :, 1], pbb[:, :, 1], g8y)
    # p_wh = (2*sig)^2 * anchor
    s.activation(out=t1[:], in_=sig[:, :, 2], func=AF.Square, scale=2.0)
    v.tensor_mul(pbb[:, :, 2], t1[:], awp)
    s.activation(out=t2[:], in_=sig[:, :, 3], func=AF.Square, scale=2.0)
    v.tensor_mul(pbb[:, :, 3], t2[:], ahp)
    sy.dma_start(out=d["pbb"].rearrange("(p f) c -> p f c", p=P), in_=pbb[:])

    # corners + area (fp32; the iw/ih cancellation needs fp32)
    hw = cp.tile([P, F], F32, tag="hw")
    hh = cp.tile([P, F], F32, tag="hh")
    v.tensor_scalar_mul(hw[:], pbb[:, :, 2], 0.5)
    v.tensor_scalar_mul(hh[:], pbb[:, :, 3], 0.5)
    ax1 = cp.tile([P, F], IOU_DT, tag="ax1")
    ax2 = cp.tile([P, F], IOU_DT, tag="ax2")
    ay1 = cp.tile([P, F], IOU_DT, tag="ay1")
    ay2 = cp.tile([P, F], IOU_DT, tag="ay2")
    v.tensor_sub(ax1[:], pbb[:, :, 0], hw[:])
    v.tensor_add(ax2[:], pbb[:, :, 0], hw[:])
    v.tensor_sub(ay1[:], pbb[:, :, 1], hh[:])
    v.tensor_add(ay2[:], pbb[:, :, 1], hh[:])
    area_b = cp.tile([P, F], IOU_DT, tag="area_b")
    v.tensor_mul(t1[:], pbb[:, :, 2], pbb[:, :, 3])
    v.tensor_copy(out=area_b[:], in_=t1[:])

    # ---------------- tconf: max IoU vs 32 GTs ----------------
    # Per position, maximize iou_g = inter_g/union_g over g. Since
    # union = S - inter with S = area + areaG, and x/(S-x) is monotone
    # increasing in x/S, maximizing iou is maximizing r = inter/S; work in
    # log domain: d = ln(inter) - ln(S). Min/max via relu identities on the
    # (otherwise idle) ScalarEngine with per-partition GT biases:
    #   iw = relu(gw - relu(bx2-ax2) - relu(ax1-bx1))
    # leaving only 5 fp16 tensor_tensor ops per GT on the VectorEngine.
    # gt fields: 0:bx2 1:-bx1 2:by2 3:-by1 4:gw 5:gh 6:areaG
    best = cp.tile([P, F], IOU_DT, tag="best")
    v.memset(best[:], -60000.0)
    epsb = cp.tile([P, 1], F32, tag="epsb")
    v.memset(epsb[:], 1e-30)
    ux = cp.tile([P, F], IOU_DT, tag="ux")
    tx = cp.tile([P, F], IOU_DT, tag="tx")
    uy = cp.tile([P, F], IOU_DT, tag="uy")
    ty = cp.tile([P, F], IOU_DT, tag="ty")
    iwb = cp.tile([P, F], IOU_DT, tag="iwb")
    ihb = cp.tile([P, F], IOU_DT, tag="ihb")
    inter = cp.tile([P, F], IOU_DT, tag="inter")
    li = cp.tile([P, F], IOU_DT, tag="li")
    ls = cp.tile([P, F], IOU_DT, tag="ls")
    for gi in range(G):
        s.activation(out=ux[:], in_=ax2[:], func=AF.Relu, scale=-1.0,
                     bias=gt[:, 0, gi:gi + 1])               # relu(bx2-ax2)
        s.activation(out=tx[:], in_=ax1[:], func=AF.Relu,
                     bias=gt[:, 1, gi:gi + 1])               # relu(ax1-bx1)
        v.tensor_add(ux[:], ux[:], tx[:])
        s.activation(out=iwb[:], in_=ux[:], func=AF.Relu, scale=-1.0,
                     bias=gt[:, 4, gi:gi + 1])               # iw
        s.activation(out=uy[:], in_=ay2[:], func=AF.Relu, scale=-1.0,
                     bias=gt[:, 2, gi:gi + 1])
        s.activation(out=ty[:], in_=ay1[:], func=AF.Relu,
                     bias=gt[:, 3, gi:gi + 1])
        v.tensor_add(uy[:], uy[:], ty[:])
        s.activation(out=ihb[:], in_=uy[:], func=AF.Relu, scale=-1.0,
                     bias=gt[:, 5, gi:gi + 1])               # ih
        v.tensor_mul(inter[:], iwb[:], ihb[:])
        s.activation(out=li[:], in_=inter[:], func=AF.Ln, bias=epsb[:])
        s.activation(out=ls[:], in_=area_b[:], func=AF.Ln,
                     bias=gt[:, 6, gi:gi + 1])               # ln(area+areaG)
        v.tensor_sub(li[:], li[:], ls[:])
        v.tensor_max(best[:], best[:], li[:])
    # tconf = r/(1-r), r = exp(best) = max_g inter/S  (r <= 0.5)
    tconf = cp.tile([P, F], F32, tag="tconf")
    rr = cp.tile([P, F], F32, tag="rr")
    s.activation(out=tconf[:], in_=best[:], func=AF.Exp)
    v.tensor_scalar(rr[:], tconf[:], -1.0, 1.0, op0=OP.mult, op1=OP.add)
    v.reciprocal(rr[:], rr[:])
    v.tensor_mul(tconf[:], tconf[:], rr[:])

    # ---------------- cls max / argmax + score ----------------
    # argmax via eq-mask * descending-iota, reduce_max picks the FIRST
    # (smallest class) among ties: idx = NCLS - max_c((x_c>=max)*(NCLS-c)).
    maxv = cp.tile([P, F], F32, tag="maxv")
    idxv = cp.tile([P, F], BF16, tag="idxv")
    iotb = cp.tile([P, NCLS], BF16, tag="iotb")
    sy.dma_start(out=iotb[:], in_=d["iot"][:])
    cls_r = d["cls"].rearrange("(p f) c -> p f c", p=P)
    for c in range(NCHUNK):
        ct = clsp.tile([P, KC, NCLS], F32, tag="clsbuf")
        sy.dma_start(out=ct[:], in_=cls_r[:, c * KC:(c + 1) * KC, :])
        msl = maxv[:, c * KC:(c + 1) * KC]
        v.reduce_max(out=msl, in_=ct[:], axis=AX.X)
        eq = clsp.tile([P, KC, NCLS], BF16, tag="eqbuf")
        v.tensor_tensor(out=eq[:], in0=ct[:],
                        in1=msl.to_broadcast([P, KC, NCLS]),
                        op=OP.is_ge)
        v.tensor_tensor(out=eq[:], in0=eq[:],
                        in1=iotb[:].rearrange("p (o c) -> p o c", o=1)
                            .to_broadcast([P, KC, NCLS]),
                        op=OP.mult)
        v.reduce_max(out=idxv[:, c * KC:(c + 1) * KC], in_=eq[:], axis=AX.X)
    idxp = cp.tile([P, F], U32, tag="idxp")
    v.tensor_scalar(idxp[:], idxv[:], -1.0, float(NCLS), op0=OP.mult, op1=OP.add)
    sy.dma_start(out=d["idx"].rearrange("(p f) -> p f", p=P), in_=idxp[:])

    smx = cp.tile([P, F], F32, tag="smx")
    s.activation(out=smx[:], in_=maxv[:], func=AF.Sigmoid)
    scf = cp.tile([P, F], F32, tag="scf")
    s.activation(out=scf[:], in_=conf[:], func=AF.Sigmoid)
    sco = cp.tile([P, F], F32, tag="sco")
    v.tensor_mul(sco[:], smx[:], scf[:])
    sy.dma_start(out=d["sco"].rearrange("(p f) -> p f", p=P), in_=sco[:])

    # ---------------- loss ----------------
    # dense part: sum softplus(conf) - sum conf*tconf
    # softplus(x) = ln(1 + e^x); safe since |conf logits| <~ 6.
    sp_acc = cp.tile([P, 1], F32, tag="sp_acc")
    spt = cp.tile([P, F], F32, tag="spt")
    s.activation(out=spt[:], in_=conf[:], func=AF.Exp)
    v.tensor_scalar_add(spt[:], spt[:], 1.0)
    s.activation(out=spt[:], in_=spt[:], func=AF.Ln, accum_out=sp_acc[:])
    v.tensor_mul(spt[:], conf[:], tconf[:])
    dot = cp.tile([P, 1], F32, tag="dot")
    v.reduce_sum(out=dot[:], in_=spt[:], axis=AX.X)

    # small losses at the 32 matched cells (gather rows via indirect DMA)
    gb4 = cp.tile([P, 4], F32, tag="gb4")
    g.indirect_dma_start(
        out=gb4[:], out_offset=None, in_=d["tbox"][:],
        in_offset=bass.IndirectOffsetOnAxis(ap=sl_idx[:, :1], axis=0))
    gb80 = cp.tile([P, NCLS], F32, tag="gb80")
    g.indirect_dma_start(
        out=gb80[:], out_offset=None, in_=d["cls"][:],
        in_offset=bass.IndirectOffsetOnAxis(ap=sl_idx[:, :1], axis=0))
    bce = cp.tile([P, 84], F32, tag="bce")
    xt = cp.tile([P, 84], F32, tag="xt")
    s.activation(out=bce[:, 0:4], in_=gb4[:], func=AF.Exp)
    s.activation(out=bce[:, 4:84], in_=gb80[:], func=AF.Exp)
    v.tensor_scalar_add(bce[:], bce[:], 1.0)
    s.activation(out=bce[:], in_=bce[:], func=AF.Ln)
    v.tensor_mul(xt[:, 0:4], gb4[:], sl_tgt[:, 0:4])
    v.tensor_mul(xt[:, 4:84], gb80[:], sl_tgt[:, 4:84])
    v.tensor_sub(bce[:], bce[:], xt[:])
    r1 = cp.tile([P, 1], F32, tag="r1")
    r2 = cp.tile([P, 1], F32, tag="r2")
    v.reduce_sum(out=r1[:], in_=bce[:, 0:4], axis=AX.X)
    v.reduce_sum(out=r2[:], in_=bce[:, 4:84], axis=AX.X)
    v.tensor_scalar_mul(r1[:], r1[:], sl_w[:, 0:1])
    v.tensor_scalar_mul(r2[:], r2[:], sl_w[:, 1:2])

    tot = cp.tile([P, 1], F32, tag="tot")
    v.tensor_sub(tot[:], sp_acc[:], dot[:])
    v.tensor_add(tot[:], tot[:], r1[:])
    v.tensor_add(tot[:], tot[:], r2[:])
    ones = cp.tile([P, 1], F32, tag="ones")
    v.memset(ones[:], 1.0)
    lps = pp.tile([1, 1], F32, tag="lps")
    nc.tensor.matmul(out=lps[:], lhsT=tot[:], rhs=ones[:], start=True, stop=True)
    lsb = cp.tile([1, 1], F32, tag="lsb")
    v.tensor_copy(out=lsb[:], in_=lps[:])
    sy.dma_start(out=d["los"][:], in_=lsb[:])
    ctx.close()


_NC = {}


def build_nc(reps=1):
    if reps in _NC:
        return _NC[reps]
    nc = bacc.Bacc("TRN2", target_bir_lowering=False)
    d = {}
    d["cls"] = nc.dram_tensor("cls", [N, NCLS], F32, kind="ExternalInput").ap()
    d["tbox"] = nc.dram_tensor("tbox", [N, 4], F32, kind="ExternalInput").ap()
    d["conf"] = nc.dram_tensor("conf", [N], F32, kind="ExternalInput").ap()
    d["planes"] = nc.dram_tensor("planes", [4, N], F32, kind="ExternalInput").ap()
    d["gt"] = nc.dram_tensor("gt", [P, 7, G], F32, kind="ExternalInput").ap()
    d["iot"] = nc.dram_tensor("iot", [P, NCLS], BF16, kind="ExternalInput").ap()
    d["sl_idx"] = nc.dram_tensor("sl_idx", [P, 1], I32, kind="ExternalInput").ap()
    d["sl_tgt"] = nc.dram_tensor("sl_tgt", [P, 84], F32, kind="ExternalInput").ap()
    d["sl_w"] = nc.dram_tensor("sl_w", [P, 2], F32, kind="ExternalInput").ap()
    d["pbb"] = nc.dram_tensor("pbb", [N, 4], F32, kind="ExternalOutput").ap()
    d["idx"] = nc.dram_tensor("idx", [N], U32, kind="ExternalOutput").ap()
    d["sco"] = nc.dram_tensor("sco", [N], F32, kind="ExternalOutput").ap()
    d["los"] = nc.dram_tensor("los", [1, 1], F32, kind="ExternalOutput").ap()
    with tile.TileContext(nc) as tc:
        for r in range(reps):
            _emit(nc, tc, d, sfx=f"r{r}" if r else "")
    nc.compile()
    _NC[reps] = nc
    return nc


def make_in_maps(t_bbox, conf_logits, cls_logits, gt_bboxes, gt_cls, gt_mask):
    planes = _position_planes()
    t_bbox = np.asarray(t_bbox, dtype=np.float32)
    conf_logits = np.asarray(conf_logits, dtype=np.float32)
    cls_logits = np.asarray(cls_logits, dtype=np.float32)
    gt_bboxes = np.asarray(gt_bboxes, dtype=np.float32)
    gt_cls = np.asarray(gt_cls)
    gt_mask = np.asarray(gt_mask)

    # ---- host-side per-GT matching (tiny, mirrors reference exactly) ----
    gw, gh = gt_bboxes[..., 2], gt_bboxes[..., 3]          # [nB,G]
    aw, ah = ANCHORS_ALL[:, 0], ANCHORS_ALL[:, 1]          # [9]
    inter = np.minimum(gw[..., None], aw) * np.minimum(gh[..., None], ah)
    union = (gw * gh)[..., None] + aw * ah - inter
    anch_idx = np.argmax(inter / (union + 1e-16), axis=-1)  # [nB,G]
    matched = anch_idx < NA
    valid = gt_mask & matched
    ta = (anch_idx % NA).astype(np.int64)
    ti = np.clip((gt_bboxes[..., 0] / STRIDE).astype(np.int32), 0, NW - 1)
    tj = np.clip((gt_bboxes[..., 1] / STRIDE).astype(np.int32), 0, NH - 1)
    flat = (ta * (NH * NW) + tj.astype(np.int64) * NW + ti).astype(np.int32)
    tgt_xy = ((gt_bboxes[..., 0:2] / STRIDE) % 1.0 + 0.5) / 2.0
    tgt_wh = np.sqrt(gt_bboxes[..., 2:4] / ANCH_LVL[ta]) / 2.0
    onehot = (gt_cls[..., None] == np.arange(NCLS)).astype(np.float32)
    vm = valid.astype(np.float32)

    # GT corner boxes for tconf IoU; invalid GTs -> far-away unit boxes so
    # their IoU is exactly 0 (matches reference's mask->-1 + has_gt logic).
    cx, cy = gt_bboxes[..., 0], gt_bboxes[..., 1]
    bx1 = cx - gw * 0.5
    bx2 = cx + gw * 0.5
    by1 = cy - gh * 0.5
    by2 = cy + gh * 0.5
    ag = gw * gh
    gww = bx2 - bx1
    ghh = by2 - by1
    m = gt_mask.astype(bool)
    # far enough to never overlap real boxes, small enough for fp16
    FARV = np.float32(3e4)
    bx1 = np.where(m, bx1, FARV)
    bx2 = np.where(m, bx2, FARV + 1)
    by1 = np.where(m, by1, FARV)
    by2 = np.where(m, by2, FARV + 1)
    ag = np.where(m, ag, np.float32(1.0))
    gww = np.where(m, gww, np.float32(1.0))
    ghh = np.where(m, ghh, np.float32(1.0))

    import ml_dtypes
    iota_desc = np.broadcast_to(
        (NCLS - np.arange(NCLS, dtype=np.float32))[None], (P, NCLS)
    ).astype(ml_dtypes.bfloat16)

    in_maps = []
    for b in range(NCORES):
        gt5 = np.stack([bx2[b], -bx1[b], by2[b], -by1[b],
                        gww[b], ghh[b], ag[b]]).astype(np.float32)
        gt_t = np.broadcast_to(gt5[None], (P, 7, G)).copy()
        sl_idx = np.zeros((P, 1), np.int32)
        sl_idx[:G, 0] = np.where(valid[b], flat[b], 0)
        sl_tgt = np.zeros((P, 84), np.float32)
        sl_tgt[:G, 0:2] = tgt_xy[b]
        sl_tgt[:G, 2:4] = tgt_wh[b]
        sl_tgt[:G, 4:84] = onehot[b]
        sl_w = np.zeros((P, 2), np.float32)
        sl_w[:G, 0] = vm[b]
        sl_w[:G, 1] = vm[b] / np.float32(NCLS)
        in_maps.append(dict(
            cls=np.ascontiguousarray(cls_logits[b].reshape(N, NCLS)),
            tbox=np.ascontiguousarray(t_bbox[b].reshape(N, 4)),
            conf=np.ascontiguousarray(conf_logits[b].reshape(N)),
            planes=planes,
            gt=gt_t,
            iot=iota_desc,
            sl_idx=sl_idx,
            sl_tgt=sl_tgt,
            sl_w=sl_w,
        ))
    return in_maps


def kernel(t_bbox, conf_logits, cls_logits, gt_bboxes, gt_cls, gt_mask):
    nc = build_nc()
    in_maps = make_in_maps(t_bbox, conf_logits, cls_logits,
                           gt_bboxes, gt_cls, gt_mask)
    res = run_bass_kernel_spmd(nc, in_maps, list(range(NCORES)))
    rs = res.results
    p_bbox = np.stack([rs[b]["pbb"] for b in range(NCORES)])
    cls_idx = np.stack([rs[b]["idx"].view(np.int32) for b in range(NCORES)])
    score = np.stack([rs[b]["sco"] for b in range(NCORES)])
    loss = np.float32(sum(float(rs[b]["los"][0, 0]) for b in range(NCORES)) / NCORES)
    return p_bbox, cls_idx, score, loss
